# revision 34
# baseline (speedup 1.0000x reference)
"""Trainium2 Bass kernel for nn_CESLayer: y = cos((x+1)*30 @ theta.T + phi).

Math: (x+1)*30 @ theta.T + phi = x @ W + bias, with
  W[k, o] = 30 * theta[o, k],  bias[o] = 30 * sum_k theta[o, k] + phi[o]
and cos(z) = sin(z + pi/2). The ScalarE Sin LUT is only valid on [-pi, pi],
so the kernel computes u = z / (2*pi) via a rescaled matmul (W' = W/2pi,
bias'/2pi folded in as a rank-1 K=1 matmul), range-reduces n = rtne(u) with
the f32 magic-number trick on VectorE, subtracts n, and evaluates
Sin(2*pi*f + residual_bias) on ScalarE.

The n-subtraction alternates between two equivalent forms to balance engine
load ("hybrid"): a VectorE tensor_tensor subtract (DVE-heavy) and a PE
accumulation of (-I) @ n into PSUM (PE-heavy; n is a small integer, exact
in bf16).

Layout: the output is computed TRANSPOSED (psum tiles are [o, b]) so the
per-output bias rides per-partition scalar operands; the host un-transposes.
x is pre-transposed host-side into [k, b] super-group tiles of 4 batch
groups so every DMA is a single contiguous 512KB block with 4KB partition
rows (per-packet DMA overhead amortized). Batch is split across 8
NeuronCores (data parallel), weights replicated.

Matmuls run in fp16 (full PE rate vs 4x slower fp32) with fp32 PSUM
accumulation; the output is stored fp16 (upcast on host), halving write
traffic. fp16 quantization of x/W contributes ~5e-4 relative error.
"""

import os
import sys

for _p in (
    "/root/.axon_site",
    "/root/.axon_site/_ro/trn_rl_repo",
    "/root/.axon_site/_ro/pypackages",
    "/opt/trn_rl_repo",
):
    if os.path.isdir(_p) and _p not in sys.path:
        sys.path.append(_p)

import ml_dtypes
import numpy as np

OMEGA_0 = 30.0
B, IN_DIM, OUT_DIM = 131072, 512, 512
N_CORES = 8
BS = B // N_CORES  # rows per core
P = 128  # partitions
KB = IN_DIM // P  # contraction blocks
OB = OUT_DIM // P  # output blocks
GW = 512  # batch columns per matmul group (moving free dim)
SG = 4  # batch groups per DMA super-group
GROUPS = BS // GW
SGROUPS = GROUPS // SG
SGW = SG * GW
MAGIC = float(np.float32(1.5 * 2**23))  # f32 round-to-nearest via add/sub
TWO_PI = 2.0 * np.pi

# mm_dt: matmul operand dtype; out_dt: DMA'd output dtype;
# epilogue: "magic" (DVE subtract), "magic2" (PE -I@n subtract), "hybrid"
CONFIG = {
    "mm_dt": os.environ.get("K_MM_DT", "f16"),
    "out_dt": os.environ.get("K_OUT_DT", "f16"),
    "epilogue": os.environ.get("K_EPILOGUE", "hybrid"),
}

_cache = {}


def _np_dt(name):
    return {
        "f16": np.float16,
        "bf16": ml_dtypes.bfloat16,
        "f32": np.float32,
        "f32r": np.float32,
    }[name]


def _build(sgroups=SGROUPS, num_devices=N_CORES, cfg=None):
    import concourse.mybir as mybir
    import concourse.tile as tile
    from concourse import bacc

    cfg = dict(CONFIG if cfg is None else cfg)
    f32 = mybir.dt.float32
    bf16 = mybir.dt.bfloat16
    mm_dt = {
        "f16": mybir.dt.float16,
        "bf16": mybir.dt.bfloat16,
        "f32r": mybir.dt.float32r,
    }[cfg["mm_dt"]]
    out_dt = {"f16": mybir.dt.float16, "f32": mybir.dt.float32}[cfg["out_dt"]]
    Alu = mybir.AluOpType
    Act = mybir.ActivationFunctionType

    nc = bacc.Bacc(
        "TRN2",
        target_bir_lowering=False,
        debug=False,
        enable_asserts=False,
        num_devices=num_devices,
    )
    # xt[gs, k, s*GW + b] = x[(gs*SG + s)*GW + b, k]
    xt_d = nc.dram_tensor(
        "xt", [sgroups, IN_DIM, SGW], mm_dt, kind="ExternalInput"
    ).ap()
    # w[k, o] = 30 * theta[o, k] / (2*pi)
    w_d = nc.dram_tensor("w", [IN_DIM, OUT_DIM], mm_dt, kind="ExternalInput").ap()
    # bias'/2pi (mod 1), one rank-1 matmul row; residual compensated via bias2
    bias_row_d = nc.dram_tensor(
        "bias_row", [1, OUT_DIM], mm_dt, kind="ExternalInput"
    ).ap()
    ones_d = nc.dram_tensor("ones_row", [1, GW], mm_dt, kind="ExternalInput").ap()
    bias2_d = nc.dram_tensor("bias2", [P, OB], f32, kind="ExternalInput").ap()
    negi_d = nc.dram_tensor("neg_ident", [P, P], bf16, kind="ExternalInput").ap()
    # yt[ob, gs, p, s*GW + b] = y[(gs*SG + s)*GW + b, ob*P + p]
    yt_d = nc.dram_tensor(
        "yt", [OB, sgroups, P, SGW], out_dt, kind="ExternalOutput"
    ).ap()

    with tile.TileContext(nc) as tc:
        with (
            tc.tile_pool(name="const", bufs=1) as cpool,
            tc.tile_pool(name="xin", bufs=2) as xpool,
            tc.tile_pool(name="eps", bufs=4) as epool,
            tc.tile_pool(name="yout", bufs=2) as ypool,
            tc.tile_pool(name="psumM", bufs=8, space="PSUM") as pMpool,
        ):
            w_sb = cpool.tile([P, KB * OUT_DIM], mm_dt)
            for j in range(KB):
                nc.sync.dma_start(
                    w_sb[:, j * OUT_DIM : (j + 1) * OUT_DIM],
                    w_d[j * P : (j + 1) * P, :],
                )
            bias2_sb = cpool.tile([P, OB], f32)
            nc.sync.dma_start(bias2_sb[:], bias2_d[:])
            biasrow_sb = cpool.tile([1, OUT_DIM], mm_dt)
            nc.sync.dma_start(biasrow_sb[:], bias_row_d[:])
            ones_sb = cpool.tile([1, GW], mm_dt)
            nc.sync.dma_start(ones_sb[:], ones_d[:])
            negi_sb = cpool.tile([P, P], bf16)
            nc.sync.dma_start(negi_sb[:], negi_d[:])

            _pending = None

            def _flush_pending(pend):
                pm_, n_, ys_, ob_ = pend
                nc.tensor.matmul(
                    pm_[:],
                    negi_sb[:],
                    n_[:],
                    start=False,
                    stop=True,
                    skip_group_check=True,
                )
                nc.scalar.activation(
                    ys_,
                    pm_[:],
                    Act.Sin,
                    scale=float(TWO_PI),
                    bias=bias2_sb[:, ob_ : ob_ + 1],
                )

            for gs in range(sgroups):
                xg = xpool.tile([P, KB * SGW], mm_dt)
                if gs == 0:
                    # fine-grained loads so the first matmul doesn't wait for
                    # the whole 2MB super-group
                    for s in range(SG):
                        for j in range(KB):
                            nc.sync.dma_start(
                                xg[:, j * SGW + s * GW : j * SGW + (s + 1) * GW],
                                xt_d[gs, j * P : (j + 1) * P, s * GW : (s + 1) * GW],
                            )
                else:
                    for j in range(KB):
                        nc.sync.dma_start(
                            xg[:, j * SGW : (j + 1) * SGW],
                            xt_d[gs, j * P : (j + 1) * P, :],
                        )
                ysw = [
                    ypool.tile(
                        [P, SGW], out_dt, tag=f"ys{ob}", name=f"ysw{ob}_{gs}"
                    )
                    for ob in range(OB)
                ]
                for ob in range(OB):
                    for s in range(SG):
                        pm = pMpool.tile([P, GW], f32)
                        nc.tensor.matmul(
                            pm[:],
                            biasrow_sb[:, ob * P : (ob + 1) * P],
                            ones_sb[:],
                            start=True,
                            stop=False,
                        )
                        for j in range(KB):
                            nc.tensor.matmul(
                                pm[:],
                                w_sb[
                                    :,
                                    j * OUT_DIM + ob * P : j * OUT_DIM + (ob + 1) * P,
                                ],
                                xg[:, j * SGW + s * GW : j * SGW + (s + 1) * GW],
                                start=False,
                                stop=(j == KB - 1),
                            )
                        # flush the previous PE-path tile: by now its TS has
                        # had a full tile of main matmuls to complete, so the
                        # in-order PE queue doesn't stall on the -I @ n matmul
                        if _pending is not None:
                            _flush_pending(_pending)
                            _pending = None
                        ys = ysw[ob][:, s * GW : (s + 1) * GW]
                        if cfg["epilogue"] == "hybrid":
                            use_pe = (s + ob) % 4 == 0
                        else:
                            use_pe = cfg["epilogue"] == "magic2"
                        if use_pe:
                            # n on DVE (bf16 exact for the small integer),
                            # -I @ n accumulated by PE, ACT reads PSUM
                            n_t = epool.tile([P, GW], bf16, tag="nb")
                            nc.vector.tensor_scalar(
                                n_t[:], pm[:], MAGIC, MAGIC, Alu.add, Alu.subtract
                            )
                            _pending = (pm, n_t, ys, ob)
                        else:
                            n_t = epool.tile([P, GW], f32, tag="nf")
                            nc.vector.tensor_scalar(
                                n_t[:], pm[:], MAGIC, MAGIC, Alu.add, Alu.subtract
                            )
                            f_t = epool.tile([P, GW], f32, tag="ff")
                            nc.vector.tensor_tensor(
                                f_t[:], pm[:], n_t[:], Alu.subtract
                            )
                            nc.scalar.activation(
                                ys,
                                f_t[:],
                                Act.Sin,
                                scale=float(TWO_PI),
                                bias=bias2_sb[:, ob : ob + 1],
                            )
                    # strip ob complete (modulo one possibly-pending tile):
                    # ship it while later strips compute
                    if _pending is not None and _pending[3] == ob:
                        _flush_pending(_pending)
                        _pending = None
                    nc.sync.dma_start(yt_d[ob, gs], ysw[ob][:])

    nc.compile()
    return nc


def _get_nc():
    if "nc" not in _cache:
        _cache["nc"] = _build()
    return _cache["nc"]


def _host_params(theta, phi, cfg=None):
    cfg = dict(CONFIG if cfg is None else cfg)
    mm_np = _np_dt(cfg["mm_dt"])
    w = np.ascontiguousarray(
        (OMEGA_0 / TWO_PI) * theta.T.astype(np.float64)
    ).astype(mm_np)
    bias = (
        (OMEGA_0 * theta.astype(np.float64).sum(axis=1) + phi + np.pi / 2) / TWO_PI
    ).astype(np.float32)
    # fold bias into the matmul mod 1 so the low-precision row stays accurate;
    # the dtype-rounding residual is re-applied in the ACT bias (bias2)
    bias_red = (bias - np.round(bias.astype(np.float64))).astype(np.float32)
    bias_row = np.ascontiguousarray(bias_red.reshape(1, OUT_DIM)).astype(mm_np)
    bias2 = np.ascontiguousarray(
        (TWO_PI * (bias_red - bias_row.astype(np.float32))).reshape(OB, P).T
    ).astype(np.float32)
    return w, bias_row, bias2


def _pretranspose(x_shard, sgroups=SGROUPS, cfg=None):
    cfg = dict(CONFIG if cfg is None else cfg)
    mm_np = _np_dt(cfg["mm_dt"])
    x5 = x_shard.astype(mm_np).reshape(sgroups, SGW, IN_DIM)
    return np.ascontiguousarray(x5.transpose(0, 2, 1))


def kernel(x, theta, phi, **run_kwargs):
    from concourse import bass_utils

    nc = _get_nc()
    w, bias_row, bias2 = _host_params(theta, phi)
    ones_row = np.ones((1, GW), _np_dt(CONFIG["mm_dt"]))
    neg_ident = (-np.eye(P, dtype=np.float32)).astype(ml_dtypes.bfloat16)

    in_maps = [
        {
            "xt": _pretranspose(x[c * BS : (c + 1) * BS]),
            "w": w,
            "bias_row": bias_row,
            "bias2": bias2,
            "ones_row": ones_row,
            "neg_ident": neg_ident,
        }
        for c in range(N_CORES)
    ]
    res = bass_utils.run_bass_kernel_spmd(
        nc, in_maps, core_ids=list(range(N_CORES)), **run_kwargs
    )
    # yt[ob, gs, p, s*GW+b] -> y[(gs*SG+s)*GW+b, ob*P+p]
    y = np.concatenate(
        [
            res.results[c]["yt"].transpose(1, 3, 0, 2).reshape(BS, OUT_DIM)
            for c in range(N_CORES)
        ],
        axis=0,
    ).astype(np.float32)
    if run_kwargs:
        _cache["last_results"] = res
    return y


# revision 36
# speedup vs baseline: 1.0225x; 1.0225x over previous
"""Trainium2 Bass kernel for nn_CESLayer: y = cos((x+1)*30 @ theta.T + phi).

Math: (x+1)*30 @ theta.T + phi = x @ W + bias, with
  W[k, o] = 30 * theta[o, k],  bias[o] = 30 * sum_k theta[o, k] + phi[o]
and cos(z) = sin(z + pi/2). The ScalarE Sin LUT is only valid on [-pi, pi],
so the kernel computes u = z / (2*pi) via a rescaled matmul (W' = W/2pi,
bias'/2pi folded in as a rank-1 K=1 matmul), range-reduces n = rtne(u) with
the f32 magic-number trick on VectorE, subtracts n, and evaluates
Sin(2*pi*f + residual_bias) on ScalarE.

The n-subtraction alternates between two equivalent forms to balance engine
load ("hybrid"): a VectorE tensor_tensor subtract (DVE-heavy) and a PE
accumulation of (-I) @ n into PSUM (PE-heavy; n is a small integer, exact
in bf16).

Layout: the output is computed TRANSPOSED (psum tiles are [o, b]) so the
per-output bias rides per-partition scalar operands; the host un-transposes.
x is pre-transposed host-side into [k, b] super-group tiles of 4 batch
groups so every DMA is a single contiguous 512KB block with 4KB partition
rows (per-packet DMA overhead amortized). Batch is split across 8
NeuronCores (data parallel), weights replicated.

Matmuls run in fp16 (full PE rate vs 4x slower fp32) with fp32 PSUM
accumulation; the output is stored fp16 (upcast on host), halving write
traffic. fp16 quantization of x/W contributes ~5e-4 relative error.
"""

import os
import sys

for _p in (
    "/root/.axon_site",
    "/root/.axon_site/_ro/trn_rl_repo",
    "/root/.axon_site/_ro/pypackages",
    "/opt/trn_rl_repo",
):
    if os.path.isdir(_p) and _p not in sys.path:
        sys.path.append(_p)

import ml_dtypes
import numpy as np

OMEGA_0 = 30.0
B, IN_DIM, OUT_DIM = 131072, 512, 512
N_CORES = 8
BS = B // N_CORES  # rows per core
P = 128  # partitions
KB = IN_DIM // P  # contraction blocks
OB = OUT_DIM // P  # output blocks
GW = 512  # batch columns per matmul group (moving free dim)
SG = 4  # batch groups per DMA super-group
GROUPS = BS // GW
SGROUPS = GROUPS // SG
SGW = SG * GW
MAGIC = float(np.float32(1.5 * 2**23))  # f32 round-to-nearest via add/sub
TWO_PI = 2.0 * np.pi

# mm_dt: matmul operand dtype; out_dt: DMA'd output dtype;
# epilogue: "magic" (DVE subtract), "magic2" (PE -I@n subtract), "hybrid"
CONFIG = {
    "mm_dt": os.environ.get("K_MM_DT", "f16"),
    "out_dt": os.environ.get("K_OUT_DT", "f16"),
    "epilogue": os.environ.get("K_EPILOGUE", "hybrid"),
}

_cache = {}


def _np_dt(name):
    return {
        "f16": np.float16,
        "bf16": ml_dtypes.bfloat16,
        "f32": np.float32,
        "f32r": np.float32,
    }[name]


def _build(sgroups=SGROUPS, num_devices=N_CORES, cfg=None):
    import concourse.mybir as mybir
    import concourse.tile as tile
    from concourse import bacc

    cfg = dict(CONFIG if cfg is None else cfg)
    f32 = mybir.dt.float32
    bf16 = mybir.dt.bfloat16
    mm_dt = {
        "f16": mybir.dt.float16,
        "bf16": mybir.dt.bfloat16,
        "f32r": mybir.dt.float32r,
    }[cfg["mm_dt"]]
    out_dt = {"f16": mybir.dt.float16, "f32": mybir.dt.float32}[cfg["out_dt"]]
    Alu = mybir.AluOpType
    Act = mybir.ActivationFunctionType

    nc = bacc.Bacc(
        "TRN2",
        target_bir_lowering=False,
        debug=False,
        enable_asserts=False,
        num_devices=num_devices,
    )
    # xt[gs, k, s*GW + b] = x[(gs*SG + s)*GW + b, k]
    xt_d = nc.dram_tensor(
        "xt", [sgroups, IN_DIM, SGW], mm_dt, kind="ExternalInput"
    ).ap()
    # w[k, o] = 30 * theta[o, k] / (2*pi)
    w_d = nc.dram_tensor("w", [IN_DIM, OUT_DIM], mm_dt, kind="ExternalInput").ap()
    # bias'/2pi (mod 1), one rank-1 matmul row; residual compensated via bias2
    bias_row_d = nc.dram_tensor(
        "bias_row", [1, OUT_DIM], mm_dt, kind="ExternalInput"
    ).ap()
    ones_d = nc.dram_tensor("ones_row", [1, GW], mm_dt, kind="ExternalInput").ap()
    bias2_d = nc.dram_tensor("bias2", [P, OB], f32, kind="ExternalInput").ap()
    negi_d = nc.dram_tensor("neg_ident", [P, P], bf16, kind="ExternalInput").ap()
    # yt[ob, gs, p, s*GW + b] = y[(gs*SG + s)*GW + b, ob*P + p]
    yt_d = nc.dram_tensor(
        "yt", [OB, sgroups, P, SGW], out_dt, kind="ExternalOutput"
    ).ap()

    with tile.TileContext(nc) as tc:
        with (
            tc.tile_pool(name="const", bufs=1) as cpool,
            tc.tile_pool(name="xin", bufs=2) as xpool,
            tc.tile_pool(name="eps", bufs=4) as epool,
            tc.tile_pool(name="yout", bufs=2) as ypool,
            tc.tile_pool(name="psumM", bufs=2, space="PSUM") as pMpool,
        ):
            w_sb = cpool.tile([P, KB * OUT_DIM], mm_dt)
            for j in range(KB):
                nc.sync.dma_start(
                    w_sb[:, j * OUT_DIM : (j + 1) * OUT_DIM],
                    w_d[j * P : (j + 1) * P, :],
                )
            bias2_sb = cpool.tile([P, OB], f32)
            nc.sync.dma_start(bias2_sb[:], bias2_d[:])
            biasrow_sb = cpool.tile([1, OUT_DIM], mm_dt)
            nc.sync.dma_start(biasrow_sb[:], bias_row_d[:])
            ones_sb = cpool.tile([1, GW], mm_dt)
            nc.sync.dma_start(ones_sb[:], ones_d[:])
            negi_sb = cpool.tile([P, P], bf16)
            nc.sync.dma_start(negi_sb[:], negi_d[:])

            _pending = None

            def _flush_pending(pend):
                pm_, n_, ys_, ob_ = pend
                nc.tensor.matmul(
                    pm_[:],
                    negi_sb[:],
                    n_[:],
                    start=False,
                    stop=True,
                    skip_group_check=True,
                )
                nc.scalar.activation(
                    ys_,
                    pm_[:],
                    Act.Sin,
                    scale=float(TWO_PI),
                    bias=bias2_sb[:, ob_ : ob_ + 1],
                )

            for gs in range(sgroups):
                xg = xpool.tile([P, KB * SGW], mm_dt)
                if gs == 0:
                    # fine-grained loads so the first matmul doesn't wait for
                    # the whole 2MB super-group
                    for s in range(SG):
                        for j in range(KB):
                            nc.sync.dma_start(
                                xg[:, j * SGW + s * GW : j * SGW + (s + 1) * GW],
                                xt_d[gs, j * P : (j + 1) * P, s * GW : (s + 1) * GW],
                            )
                else:
                    for j in range(KB):
                        nc.sync.dma_start(
                            xg[:, j * SGW : (j + 1) * SGW],
                            xt_d[gs, j * P : (j + 1) * P, :],
                        )
                ysw = [
                    ypool.tile(
                        [P, SGW], out_dt, tag=f"ys{ob}", name=f"ysw{ob}_{gs}"
                    )
                    for ob in range(OB)
                ]
                for ob in range(OB):
                    # j-outer / s-inner: 4 consecutive matmuls share the same
                    # stationary operand (rotating PSUM banks), which streams
                    # at full PE rate
                    pms = [
                        pMpool.tile([P, GW], f32, tag=f"pm{s}", name=f"pm{ob}_{gs}_{s}")
                        for s in range(SG)
                    ]
                    for s in range(SG):
                        nc.tensor.matmul(
                            pms[s][:],
                            biasrow_sb[:, ob * P : (ob + 1) * P],
                            ones_sb[:],
                            start=True,
                            stop=False,
                        )
                    for j in range(KB):
                        for s in range(SG):
                            nc.tensor.matmul(
                                pms[s][:],
                                w_sb[
                                    :,
                                    j * OUT_DIM + ob * P : j * OUT_DIM + (ob + 1) * P,
                                ],
                                xg[:, j * SGW + s * GW : j * SGW + (s + 1) * GW],
                                start=False,
                                stop=(j == KB - 1),
                            )
                    for s in range(SG):
                        pm = pms[s]
                        ys = ysw[ob][:, s * GW : (s + 1) * GW]
                        if cfg["epilogue"] == "hybrid":
                            use_pe = s == 0
                        else:
                            use_pe = cfg["epilogue"] == "magic2"
                        if use_pe:
                            # n on DVE (bf16 exact for the small integer),
                            # -I @ n accumulated by PE, ACT reads PSUM
                            n_t = epool.tile([P, GW], bf16, tag="nb")
                            nc.vector.tensor_scalar(
                                n_t[:], pm[:], MAGIC, MAGIC, Alu.add, Alu.subtract
                            )
                            _pending = (pm, n_t, ys, ob)
                        else:
                            n_t = epool.tile([P, GW], f32, tag="nf")
                            nc.vector.tensor_scalar(
                                n_t[:], pm[:], MAGIC, MAGIC, Alu.add, Alu.subtract
                            )
                            f_t = epool.tile([P, GW], f32, tag="ff")
                            nc.vector.tensor_tensor(
                                f_t[:], pm[:], n_t[:], Alu.subtract
                            )
                            nc.scalar.activation(
                                ys,
                                f_t[:],
                                Act.Sin,
                                scale=float(TWO_PI),
                                bias=bias2_sb[:, ob : ob + 1],
                            )
                    # ship the strip while later strips compute
                    if _pending is not None and _pending[3] == ob:
                        _flush_pending(_pending)
                        _pending = None
                    nc.sync.dma_start(yt_d[ob, gs], ysw[ob][:])

    nc.compile()
    return nc


def _get_nc():
    if "nc" not in _cache:
        _cache["nc"] = _build()
    return _cache["nc"]


def _host_params(theta, phi, cfg=None):
    cfg = dict(CONFIG if cfg is None else cfg)
    mm_np = _np_dt(cfg["mm_dt"])
    w = np.ascontiguousarray(
        (OMEGA_0 / TWO_PI) * theta.T.astype(np.float64)
    ).astype(mm_np)
    bias = (
        (OMEGA_0 * theta.astype(np.float64).sum(axis=1) + phi + np.pi / 2) / TWO_PI
    ).astype(np.float32)
    # fold bias into the matmul mod 1 so the low-precision row stays accurate;
    # the dtype-rounding residual is re-applied in the ACT bias (bias2)
    bias_red = (bias - np.round(bias.astype(np.float64))).astype(np.float32)
    bias_row = np.ascontiguousarray(bias_red.reshape(1, OUT_DIM)).astype(mm_np)
    bias2 = np.ascontiguousarray(
        (TWO_PI * (bias_red - bias_row.astype(np.float32))).reshape(OB, P).T
    ).astype(np.float32)
    return w, bias_row, bias2


def _pretranspose(x_shard, sgroups=SGROUPS, cfg=None):
    cfg = dict(CONFIG if cfg is None else cfg)
    mm_np = _np_dt(cfg["mm_dt"])
    x5 = x_shard.astype(mm_np).reshape(sgroups, SGW, IN_DIM)
    return np.ascontiguousarray(x5.transpose(0, 2, 1))


def kernel(x, theta, phi, **run_kwargs):
    from concourse import bass_utils

    nc = _get_nc()
    w, bias_row, bias2 = _host_params(theta, phi)
    ones_row = np.ones((1, GW), _np_dt(CONFIG["mm_dt"]))
    neg_ident = (-np.eye(P, dtype=np.float32)).astype(ml_dtypes.bfloat16)

    in_maps = [
        {
            "xt": _pretranspose(x[c * BS : (c + 1) * BS]),
            "w": w,
            "bias_row": bias_row,
            "bias2": bias2,
            "ones_row": ones_row,
            "neg_ident": neg_ident,
        }
        for c in range(N_CORES)
    ]
    res = bass_utils.run_bass_kernel_spmd(
        nc, in_maps, core_ids=list(range(N_CORES)), **run_kwargs
    )
    # yt[ob, gs, p, s*GW+b] -> y[(gs*SG+s)*GW+b, ob*P+p]
    y = np.concatenate(
        [
            res.results[c]["yt"].transpose(1, 3, 0, 2).reshape(BS, OUT_DIM)
            for c in range(N_CORES)
        ],
        axis=0,
    ).astype(np.float32)
    if run_kwargs:
        _cache["last_results"] = res
    return y


# revision 41
# speedup vs baseline: 1.1201x; 1.0954x over previous
"""Trainium2 Bass kernel for nn_CESLayer: y = cos((x+1)*30 @ theta.T + phi).

Math: (x+1)*30 @ theta.T + phi = x @ W + bias, with
  W[k, o] = 30 * theta[o, k],  bias[o] = 30 * sum_k theta[o, k] + phi[o]
and cos(z) = sin(z + pi/2). The ScalarE Sin LUT is only valid on [-pi, pi],
so the kernel computes u = z / (2*pi) via a rescaled matmul (W' = W/2pi,
bias'/2pi folded in as a rank-1 K=1 matmul), range-reduces n = rtne(u) with
the f32 magic-number trick on VectorE, subtracts n, and evaluates
Sin(2*pi*f + residual_bias) on ScalarE.

The n-subtraction alternates between two equivalent forms to balance engine
load ("hybrid"): a VectorE tensor_tensor subtract (DVE-heavy) and a PE
accumulation of (-I) @ n into PSUM (PE-heavy; n is a small integer, exact
in bf16).

Layout: the output is computed TRANSPOSED (psum tiles are [o, b]) so the
per-output bias rides per-partition scalar operands; the host un-transposes.
x is pre-transposed host-side into [k, b] super-group tiles of 4 batch
groups so every DMA is a single contiguous 512KB block with 4KB partition
rows (per-packet DMA overhead amortized). Batch is split across 8
NeuronCores (data parallel), weights replicated.

Matmuls run in fp16 (full PE rate vs 4x slower fp32) with fp32 PSUM
accumulation; the output is stored fp16 (upcast on host), halving write
traffic. fp16 quantization of x/W contributes ~5e-4 relative error.
"""

import os
import sys

for _p in (
    "/root/.axon_site",
    "/root/.axon_site/_ro/trn_rl_repo",
    "/root/.axon_site/_ro/pypackages",
    "/opt/trn_rl_repo",
):
    if os.path.isdir(_p) and _p not in sys.path:
        sys.path.append(_p)

import ml_dtypes
import numpy as np

OMEGA_0 = 30.0
B, IN_DIM, OUT_DIM = 131072, 512, 512
N_CORES = 8
BS = B // N_CORES  # rows per core
P = 128  # partitions
KB = IN_DIM // P  # contraction blocks
OB = OUT_DIM // P  # output blocks
GW = 512  # batch columns per matmul group (moving free dim)
SG = 4  # batch groups per DMA super-group
GROUPS = BS // GW
SGROUPS = GROUPS // SG
SGW = SG * GW
MAGIC = float(np.float32(1.5 * 2**23))  # f32 round-to-nearest via add/sub
TWO_PI = 2.0 * np.pi

# mm_dt: matmul operand dtype; out_dt: DMA'd output dtype;
# epilogue: "magic" (DVE subtract), "magic2" (PE -I@n subtract), "hybrid"
CONFIG = {
    "mm_dt": os.environ.get("K_MM_DT", "f16"),
    "out_dt": os.environ.get("K_OUT_DT", "f16"),
    "epilogue": os.environ.get("K_EPILOGUE", "hybrid"),
}

_cache = {}


def _np_dt(name):
    return {
        "f16": np.float16,
        "bf16": ml_dtypes.bfloat16,
        "f32": np.float32,
        "f32r": np.float32,
    }[name]


def _build(sgroups=SGROUPS, num_devices=N_CORES, cfg=None):
    import concourse.mybir as mybir
    import concourse.tile as tile
    from concourse import bacc

    cfg = dict(CONFIG if cfg is None else cfg)
    f32 = mybir.dt.float32
    bf16 = mybir.dt.bfloat16
    mm_dt = {
        "f16": mybir.dt.float16,
        "bf16": mybir.dt.bfloat16,
        "f32r": mybir.dt.float32r,
    }[cfg["mm_dt"]]
    out_dt = {"f16": mybir.dt.float16, "f32": mybir.dt.float32}[cfg["out_dt"]]
    Alu = mybir.AluOpType
    Act = mybir.ActivationFunctionType

    nc = bacc.Bacc(
        "TRN2",
        target_bir_lowering=False,
        debug=False,
        enable_asserts=False,
        num_devices=num_devices,
    )
    # xt[gs, k, s*GW + b] = x[(gs*SG + s)*GW + b, k]
    xt_d = nc.dram_tensor(
        "xt", [sgroups, IN_DIM, SGW], mm_dt, kind="ExternalInput"
    ).ap()
    # w[k, o] = 30 * theta[o, k] / (2*pi)
    w_d = nc.dram_tensor("w", [IN_DIM, OUT_DIM], mm_dt, kind="ExternalInput").ap()
    # bias'/2pi (mod 1), one rank-1 matmul row; residual compensated via bias2
    bias_row_d = nc.dram_tensor(
        "bias_row", [1, OUT_DIM], mm_dt, kind="ExternalInput"
    ).ap()
    ones_d = nc.dram_tensor("ones_row", [1, SGW], mm_dt, kind="ExternalInput").ap()
    bias2_d = nc.dram_tensor("bias2", [P, OB], f32, kind="ExternalInput").ap()
    negi_d = nc.dram_tensor("neg_ident", [P, P], bf16, kind="ExternalInput").ap()
    # yt[ob, gs, p, s*GW + b] = y[(gs*SG + s)*GW + b, ob*P + p]
    yt_d = nc.dram_tensor(
        "yt", [OB, sgroups, P, SGW], out_dt, kind="ExternalOutput"
    ).ap()

    with tile.TileContext(nc) as tc:
        with (
            tc.tile_pool(name="const", bufs=1) as cpool,
            tc.tile_pool(name="xin", bufs=2) as xpool,
            tc.tile_pool(name="eps", bufs=4) as epool,
            tc.tile_pool(name="yout", bufs=2) as ypool,
            tc.tile_pool(name="psumM", bufs=2, space="PSUM") as pMpool,
        ):
            # critical-path constants first: the first strip needs biasrow,
            # ones, and w before any matmul can retire
            biasrow_sb = cpool.tile([1, OUT_DIM], mm_dt)
            nc.sync.dma_start(biasrow_sb[:], bias_row_d[:])
            ones_sb = cpool.tile([1, SGW], mm_dt)
            nc.sync.dma_start(ones_sb[:], ones_d[:])
            w_sb = cpool.tile([P, KB * OUT_DIM], mm_dt)
            for j in range(KB):
                nc.sync.dma_start(
                    w_sb[:, j * OUT_DIM : (j + 1) * OUT_DIM],
                    w_d[j * P : (j + 1) * P, :],
                )
            bias2_sb = cpool.tile([P, OB], f32)
            nc.sync.dma_start(bias2_sb[:], bias2_d[:])
            negi_sb = cpool.tile([P, P], bf16)
            nc.sync.dma_start(negi_sb[:], negi_d[:])

            _pending = None

            def _flush_pending(pend):
                pm_, n_, ys_, ob_ = pend
                nc.tensor.matmul(
                    pm_[:],
                    negi_sb[:],
                    n_[:],
                    start=False,
                    stop=True,
                    skip_group_check=True,
                )
                nc.scalar.activation(
                    ys_,
                    pm_[:],
                    Act.Sin,
                    scale=float(TWO_PI),
                    bias=bias2_sb[:, ob_ : ob_ + 1],
                )

            for gs in range(sgroups):
                xg = xpool.tile([P, KB * SGW], mm_dt)
                if gs == 0:
                    # fine-grained loads so the first matmul doesn't wait for
                    # the whole 2MB super-group
                    for s in range(SG):
                        for j in range(KB):
                            nc.sync.dma_start(
                                xg[:, j * SGW + s * GW : j * SGW + (s + 1) * GW],
                                xt_d[gs, j * P : (j + 1) * P, s * GW : (s + 1) * GW],
                            )
                else:
                    for j in range(KB):
                        nc.sync.dma_start(
                            xg[:, j * SGW : (j + 1) * SGW],
                            xt_d[gs, j * P : (j + 1) * P, :],
                        )
                ysw = [
                    ypool.tile(
                        [P, SGW], out_dt, tag=f"ys{ob}", name=f"ysw{ob}_{gs}"
                    )
                    for ob in range(OB)
                ]
                for ob in range(OB):
                    # j-outer / s-inner: consecutive matmuls share the same
                    # stationary operand (rotating PSUM banks), streaming at
                    # full PE rate. s-pairs share one 2-bank [P, 2*GW] psum
                    # tile so the DVE/ACT epilogue runs half as many (wide)
                    # ops, amortizing per-op fixed overhead.
                    HP = SG // 2
                    pms = [
                        pMpool.tile(
                            [P, 2 * GW], f32, tag=f"pm{h}", name=f"pm{ob}_{gs}_{h}"
                        )
                        for h in range(HP)
                    ]
                    for s in range(SG):
                        nc.tensor.matmul(
                            pms[s // 2][:, (s % 2) * GW : (s % 2 + 1) * GW],
                            biasrow_sb[:, ob * P : (ob + 1) * P],
                            ones_sb[:, :GW],
                            start=True,
                            stop=False,
                            skip_group_check=True,
                        )
                    for j in range(KB):
                        for s in range(SG):
                            nc.tensor.matmul(
                                pms[s // 2][:, (s % 2) * GW : (s % 2 + 1) * GW],
                                w_sb[
                                    :,
                                    j * OUT_DIM + ob * P : j * OUT_DIM + (ob + 1) * P,
                                ],
                                xg[:, j * SGW + s * GW : j * SGW + (s + 1) * GW],
                                start=False,
                                stop=(j == KB - 1),
                                skip_group_check=True,
                            )
                    for h in range(HP):
                        pm = pms[h]
                        ys = ysw[ob][:, h * 2 * GW : (h + 1) * 2 * GW]
                        n_t = epool.tile([P, 2 * GW], f32, tag="nf")
                        nc.vector.tensor_scalar(
                            n_t[:], pm[:], MAGIC, MAGIC, Alu.add, Alu.subtract
                        )
                        f_t = epool.tile([P, 2 * GW], f32, tag="ff")
                        nc.vector.tensor_tensor(f_t[:], pm[:], n_t[:], Alu.subtract)
                        nc.scalar.activation(
                            ys,
                            f_t[:],
                            Act.Sin,
                            scale=float(TWO_PI),
                            bias=bias2_sb[:, ob : ob + 1],
                        )
                    # ship the strip while later strips compute
                    nc.sync.dma_start(yt_d[ob, gs], ysw[ob][:])

    nc.compile()
    return nc


def _get_nc():
    if "nc" not in _cache:
        _cache["nc"] = _build()
    return _cache["nc"]


def _host_params(theta, phi, cfg=None):
    cfg = dict(CONFIG if cfg is None else cfg)
    mm_np = _np_dt(cfg["mm_dt"])
    w = np.ascontiguousarray(
        (OMEGA_0 / TWO_PI) * theta.T.astype(np.float64)
    ).astype(mm_np)
    bias = (
        (OMEGA_0 * theta.astype(np.float64).sum(axis=1) + phi + np.pi / 2) / TWO_PI
    ).astype(np.float32)
    # fold bias into the matmul mod 1 so the low-precision row stays accurate;
    # the dtype-rounding residual is re-applied in the ACT bias (bias2)
    bias_red = (bias - np.round(bias.astype(np.float64))).astype(np.float32)
    bias_row = np.ascontiguousarray(bias_red.reshape(1, OUT_DIM)).astype(mm_np)
    bias2 = np.ascontiguousarray(
        (TWO_PI * (bias_red - bias_row.astype(np.float32))).reshape(OB, P).T
    ).astype(np.float32)
    return w, bias_row, bias2


def _pretranspose(x_shard, sgroups=SGROUPS, cfg=None):
    cfg = dict(CONFIG if cfg is None else cfg)
    mm_np = _np_dt(cfg["mm_dt"])
    x5 = x_shard.astype(mm_np).reshape(sgroups, SGW, IN_DIM)
    return np.ascontiguousarray(x5.transpose(0, 2, 1))


def kernel(x, theta, phi, **run_kwargs):
    from concourse import bass_utils

    nc = _get_nc()
    w, bias_row, bias2 = _host_params(theta, phi)
    ones_row = np.ones((1, SGW), _np_dt(CONFIG["mm_dt"]))
    neg_ident = (-np.eye(P, dtype=np.float32)).astype(ml_dtypes.bfloat16)

    in_maps = [
        {
            "xt": _pretranspose(x[c * BS : (c + 1) * BS]),
            "w": w,
            "bias_row": bias_row,
            "bias2": bias2,
            "ones_row": ones_row,
            "neg_ident": neg_ident,
        }
        for c in range(N_CORES)
    ]
    res = bass_utils.run_bass_kernel_spmd(
        nc, in_maps, core_ids=list(range(N_CORES)), **run_kwargs
    )
    # yt[ob, gs, p, s*GW+b] -> y[(gs*SG+s)*GW+b, ob*P+p]
    y = np.concatenate(
        [
            res.results[c]["yt"].transpose(1, 3, 0, 2).reshape(BS, OUT_DIM)
            for c in range(N_CORES)
        ],
        axis=0,
    ).astype(np.float32)
    if run_kwargs:
        _cache["last_results"] = res
    return y


# revision 46
# speedup vs baseline: 1.1217x; 1.0015x over previous
"""Trainium2 Bass kernel for nn_CESLayer: y = cos((x+1)*30 @ theta.T + phi).

Math: (x+1)*30 @ theta.T + phi = x @ W + bias, with
  W[k, o] = 30 * theta[o, k],  bias[o] = 30 * sum_k theta[o, k] + phi[o]
and cos(z) = sin(z + pi/2). The ScalarE Sin LUT is only valid on [-pi, pi],
so the kernel computes u = z / (2*pi) via a rescaled matmul (W' = W/2pi,
bias'/2pi folded in as a rank-1 K=1 matmul), range-reduces n = rtne(u) with
the f32 magic-number trick on VectorE, subtracts n, and evaluates
Sin(2*pi*f + residual_bias) on ScalarE.

The n-subtraction alternates between two equivalent forms to balance engine
load ("hybrid"): a VectorE tensor_tensor subtract (DVE-heavy) and a PE
accumulation of (-I) @ n into PSUM (PE-heavy; n is a small integer, exact
in bf16).

Layout: the output is computed TRANSPOSED (psum tiles are [o, b]) so the
per-output bias rides per-partition scalar operands; the host un-transposes.
x is pre-transposed host-side into [k, b] super-group tiles of 4 batch
groups so every DMA is a single contiguous 512KB block with 4KB partition
rows (per-packet DMA overhead amortized). Batch is split across 8
NeuronCores (data parallel), weights replicated.

Matmuls run in fp16 (full PE rate vs 4x slower fp32) with fp32 PSUM
accumulation; the output is stored fp16 (upcast on host), halving write
traffic. fp16 quantization of x/W contributes ~5e-4 relative error.
"""

import os
import sys

for _p in (
    "/root/.axon_site",
    "/root/.axon_site/_ro/trn_rl_repo",
    "/root/.axon_site/_ro/pypackages",
    "/opt/trn_rl_repo",
):
    if os.path.isdir(_p) and _p not in sys.path:
        sys.path.append(_p)

import ml_dtypes
import numpy as np

OMEGA_0 = 30.0
B, IN_DIM, OUT_DIM = 131072, 512, 512
N_CORES = 8
BS = B // N_CORES  # rows per core
P = 128  # partitions
KB = IN_DIM // P  # contraction blocks
OB = OUT_DIM // P  # output blocks
GW = 512  # batch columns per matmul group (moving free dim)
SG = 4  # batch groups per DMA super-group
GROUPS = BS // GW
SGROUPS = GROUPS // SG
SGW = SG * GW
MAGIC = float(np.float32(1.5 * 2**23))  # f32 round-to-nearest via add/sub
TWO_PI = 2.0 * np.pi

# mm_dt: matmul operand dtype; out_dt: DMA'd output dtype;
# epilogue: "magic" (DVE subtract), "magic2" (PE -I@n subtract), "hybrid"
CONFIG = {
    "mm_dt": os.environ.get("K_MM_DT", "f16"),
    "out_dt": os.environ.get("K_OUT_DT", "f16"),
    "epilogue": os.environ.get("K_EPILOGUE", "hybrid"),
}

_cache = {}


def _np_dt(name):
    return {
        "f16": np.float16,
        "bf16": ml_dtypes.bfloat16,
        "f32": np.float32,
        "f32r": np.float32,
    }[name]


def _build(sgroups=SGROUPS, num_devices=N_CORES, cfg=None):
    import concourse.mybir as mybir
    import concourse.tile as tile
    from concourse import bacc

    cfg = dict(CONFIG if cfg is None else cfg)
    f32 = mybir.dt.float32
    bf16 = mybir.dt.bfloat16
    mm_dt = {
        "f16": mybir.dt.float16,
        "bf16": mybir.dt.bfloat16,
        "f32r": mybir.dt.float32r,
    }[cfg["mm_dt"]]
    out_dt = {"f16": mybir.dt.float16, "f32": mybir.dt.float32}[cfg["out_dt"]]
    Alu = mybir.AluOpType
    Act = mybir.ActivationFunctionType

    nc = bacc.Bacc(
        "TRN2",
        target_bir_lowering=False,
        debug=False,
        enable_asserts=False,
        num_devices=num_devices,
    )
    # xt[gs, k, s*GW + b] = x[(gs*SG + s)*GW + b, k]
    xt_d = nc.dram_tensor(
        "xt", [sgroups, IN_DIM, SGW], mm_dt, kind="ExternalInput"
    ).ap()
    # w[k, o] = 30 * theta[o, k] / (2*pi)
    w_d = nc.dram_tensor("w", [IN_DIM, OUT_DIM], mm_dt, kind="ExternalInput").ap()
    # bias'/2pi (mod 1), one rank-1 matmul row; residual compensated via bias2
    bias_row_d = nc.dram_tensor(
        "bias_row", [1, OUT_DIM], mm_dt, kind="ExternalInput"
    ).ap()
    ones_d = nc.dram_tensor("ones_row", [1, SGW], mm_dt, kind="ExternalInput").ap()
    bias2_d = nc.dram_tensor("bias2", [P, OB], f32, kind="ExternalInput").ap()
    negi_d = nc.dram_tensor("neg_ident", [P, P], bf16, kind="ExternalInput").ap()
    # yt[ob, gs, p, s*GW + b] = y[(gs*SG + s)*GW + b, ob*P + p]
    yt_d = nc.dram_tensor(
        "yt", [OB, sgroups, P, SGW], out_dt, kind="ExternalOutput"
    ).ap()

    with tile.TileContext(nc) as tc:
        with (
            tc.tile_pool(name="const", bufs=1) as cpool,
            tc.tile_pool(name="xin", bufs=2) as xpool,
            tc.tile_pool(name="eps", bufs=4) as epool,
            tc.tile_pool(name="yout", bufs=2) as ypool,
            tc.tile_pool(name="psumM", bufs=2, space="PSUM") as pMpool,
        ):
            # critical-path constants first: the first strip needs biasrow,
            # ones, and w before any matmul can retire
            biasrow_sb = cpool.tile([1, OUT_DIM], mm_dt)
            nc.sync.dma_start(biasrow_sb[:], bias_row_d[:])
            ones_sb = cpool.tile([1, SGW], mm_dt)
            nc.sync.dma_start(ones_sb[:], ones_d[:])
            w_sb = cpool.tile([P, KB * OUT_DIM], mm_dt)
            for j in range(KB):
                nc.sync.dma_start(
                    w_sb[:, j * OUT_DIM : (j + 1) * OUT_DIM],
                    w_d[j * P : (j + 1) * P, :],
                )
            bias2_sb = cpool.tile([P, OB], f32)
            nc.sync.dma_start(bias2_sb[:], bias2_d[:])
            negi_sb = cpool.tile([P, P], bf16)
            nc.sync.dma_start(negi_sb[:], negi_d[:])

            _pending = None

            def _flush_pending(pend):
                pm_, n_, ys_, ob_, out_ap = pend
                for half in range(2):
                    nc.tensor.matmul(
                        pm_[:, half * GW : (half + 1) * GW],
                        negi_sb[:],
                        n_[:, half * GW : (half + 1) * GW],
                        start=False,
                        stop=True,
                        skip_group_check=True,
                    )
                nc.scalar.activation(
                    ys_,
                    pm_[:],
                    Act.Sin,
                    scale=float(TWO_PI),
                    bias=bias2_sb[:, ob_ : ob_ + 1],
                )
                nc.sync.dma_start(out_ap, ys_)

            for gs in range(sgroups):
                xg = xpool.tile([P, KB * SGW], mm_dt)
                if gs == 0:
                    # fine-grained loads so the first matmul doesn't wait for
                    # the whole 2MB super-group
                    for s in range(SG):
                        for j in range(KB):
                            nc.sync.dma_start(
                                xg[:, j * SGW + s * GW : j * SGW + (s + 1) * GW],
                                xt_d[gs, j * P : (j + 1) * P, s * GW : (s + 1) * GW],
                            )
                else:
                    for j in range(KB):
                        nc.sync.dma_start(
                            xg[:, j * SGW : (j + 1) * SGW],
                            xt_d[gs, j * P : (j + 1) * P, :],
                        )
                ysw = [
                    ypool.tile(
                        [P, SGW], out_dt, tag=f"ys{ob}", name=f"ysw{ob}_{gs}"
                    )
                    for ob in range(OB)
                ]
                for ob in range(OB):
                    # j-outer / s-inner: consecutive matmuls share the same
                    # stationary operand (rotating PSUM banks), streaming at
                    # full PE rate. s-pairs share one 2-bank [P, 2*GW] psum
                    # tile so the DVE/ACT epilogue runs half as many (wide)
                    # ops, amortizing per-op fixed overhead.
                    HP = SG // 2
                    pms = [
                        pMpool.tile(
                            [P, 2 * GW], f32, tag=f"pm{h}", name=f"pm{ob}_{gs}_{h}"
                        )
                        for h in range(HP)
                    ]
                    for s in range(SG):
                        nc.tensor.matmul(
                            pms[s // 2][:, (s % 2) * GW : (s % 2 + 1) * GW],
                            biasrow_sb[:, ob * P : (ob + 1) * P],
                            ones_sb[:, :GW],
                            start=True,
                            stop=False,
                            skip_group_check=True,
                        )
                    for j in range(KB):
                        for s in range(SG):
                            nc.tensor.matmul(
                                pms[s // 2][:, (s % 2) * GW : (s % 2 + 1) * GW],
                                w_sb[
                                    :,
                                    j * OUT_DIM + ob * P : j * OUT_DIM + (ob + 1) * P,
                                ],
                                xg[:, j * SGW + s * GW : j * SGW + (s + 1) * GW],
                                start=False,
                                stop=(j == KB - 1),
                                skip_group_check=True,
                            )
                        if j == 0 and _pending is not None:
                            # previous strip's PE-path subtract: its rtne had a
                            # full j-block of slack, so PE doesn't stall
                            _flush_pending(_pending)
                            _pending = None
                    for h in range(HP):
                        pm = pms[h]
                        ys = ysw[ob][:, h * 2 * GW : (h + 1) * 2 * GW]
                        if cfg["epilogue"] == "hybrid":
                            use_pe = h == 0 and ob == gs % 4
                        else:
                            use_pe = cfg["epilogue"] == "magic2"
                        if use_pe:
                            n_t = epool.tile([P, 2 * GW], bf16, tag="nb")
                            nc.vector.tensor_scalar(
                                n_t[:], pm[:], MAGIC, MAGIC, Alu.add, Alu.subtract
                            )
                            _pending = (
                                pm,
                                n_t,
                                ys,
                                ob,
                                yt_d[ob, gs][:, h * 2 * GW : (h + 1) * 2 * GW],
                            )
                        else:
                            n_t = epool.tile([P, 2 * GW], f32, tag="nf")
                            nc.vector.tensor_scalar(
                                n_t[:], pm[:], MAGIC, MAGIC, Alu.add, Alu.subtract
                            )
                            f_t = epool.tile([P, 2 * GW], f32, tag="ff")
                            nc.vector.tensor_tensor(
                                f_t[:], pm[:], n_t[:], Alu.subtract
                            )
                            nc.scalar.activation(
                                ys,
                                f_t[:],
                                Act.Sin,
                                scale=float(TWO_PI),
                                bias=bias2_sb[:, ob : ob + 1],
                            )
                    # ship the strip while later strips compute; a pending
                    # PE-path pair ships its own half from _flush_pending
                    if _pending is not None and _pending[3] == ob:
                        nc.sync.dma_start(
                            yt_d[ob, gs][:, 2 * GW :], ysw[ob][:, 2 * GW :]
                        )
                    else:
                        nc.sync.dma_start(yt_d[ob, gs], ysw[ob][:])
            if _pending is not None:
                _flush_pending(_pending)
                _pending = None

    nc.compile()
    return nc


def _get_nc():
    if "nc" not in _cache:
        _cache["nc"] = _build()
    return _cache["nc"]


def _host_params(theta, phi, cfg=None):
    cfg = dict(CONFIG if cfg is None else cfg)
    mm_np = _np_dt(cfg["mm_dt"])
    w = np.ascontiguousarray(
        (OMEGA_0 / TWO_PI) * theta.T.astype(np.float64)
    ).astype(mm_np)
    bias = (
        (OMEGA_0 * theta.astype(np.float64).sum(axis=1) + phi + np.pi / 2) / TWO_PI
    ).astype(np.float32)
    # fold bias into the matmul mod 1 so the low-precision row stays accurate;
    # the dtype-rounding residual is re-applied in the ACT bias (bias2)
    bias_red = (bias - np.round(bias.astype(np.float64))).astype(np.float32)
    bias_row = np.ascontiguousarray(bias_red.reshape(1, OUT_DIM)).astype(mm_np)
    bias2 = np.ascontiguousarray(
        (TWO_PI * (bias_red - bias_row.astype(np.float32))).reshape(OB, P).T
    ).astype(np.float32)
    return w, bias_row, bias2


def _pretranspose(x_shard, sgroups=SGROUPS, cfg=None):
    cfg = dict(CONFIG if cfg is None else cfg)
    mm_np = _np_dt(cfg["mm_dt"])
    x5 = x_shard.astype(mm_np).reshape(sgroups, SGW, IN_DIM)
    return np.ascontiguousarray(x5.transpose(0, 2, 1))


def kernel(x, theta, phi, **run_kwargs):
    from concourse import bass_utils

    nc = _get_nc()
    w, bias_row, bias2 = _host_params(theta, phi)
    ones_row = np.ones((1, SGW), _np_dt(CONFIG["mm_dt"]))
    neg_ident = (-np.eye(P, dtype=np.float32)).astype(ml_dtypes.bfloat16)

    in_maps = [
        {
            "xt": _pretranspose(x[c * BS : (c + 1) * BS]),
            "w": w,
            "bias_row": bias_row,
            "bias2": bias2,
            "ones_row": ones_row,
            "neg_ident": neg_ident,
        }
        for c in range(N_CORES)
    ]
    res = bass_utils.run_bass_kernel_spmd(
        nc, in_maps, core_ids=list(range(N_CORES)), **run_kwargs
    )
    # yt[ob, gs, p, s*GW+b] -> y[(gs*SG+s)*GW+b, ob*P+p]
    y = np.concatenate(
        [
            res.results[c]["yt"].transpose(1, 3, 0, 2).reshape(BS, OUT_DIM)
            for c in range(N_CORES)
        ],
        axis=0,
    ).astype(np.float32)
    if run_kwargs:
        _cache["last_results"] = res
    return y
